# revision 60
# baseline (speedup 1.0000x reference)
"""Trainium2 Bass kernel for nn_Attention_63995012710903.

Math: the reference's mask is `scores*tril - 1e10*(1-triu)`, which makes the
softmax argument (pre /sqrt(64)):
    p <  q : scores - 1e10   -> exp underflows to exactly 0 in fp32
    p == q : scores
    p >  q : 0               -> exp = 1
So attention collapses to, per (batch, head), with e_q = exp(s_qq/8),
cnt_q = n-1-q, Z_q = e_q + cnt_q:
    z[q,:] = (e_q * v[q,:] + sum_{p>q} v[p,:]) / Z_q
           = ((e_q - 1) * v[q,:] + S_incl[q,:]) / Z_q,   S_incl[q] = sum_{p>=q} v[p]
Only the DIAGONAL of the score matrix and suffix sums of V are needed; both
O(n^2) attention matmuls disappear.  out = z_flat @ W_O^T.

Sharding (8 cores): core c -> batch b=c//4, head-pair g=c%4 (heads 2g, 2g+1).
Each core computes q,k,v for its 2 heads (f-block of 128 of z_flat's 512
features), z for that block, and the PARTIAL output  z_blk @ W_O[:,blk]^T
([2048, 512]).  The unshard sums the 4 partials per batch on host (W_O is
column-sharded, per the problem's sharding hint).

Device layout: positions on partitions ("p-layout").  x^T ([512,2048]) is
prepared host-side so the d=512 contraction sits on partitions.  bf16 compute
(PSUM accumulates fp32); suffix sums via matmuls with a lower-triangular
ones matrix plus a running tile-sum R.
"""

import os
import sys

import numpy as np

for _p in ("/opt/trn_rl_repo", "/root/.axon_site/_ro/trn_rl_repo"):
    if os.path.isdir(_p) and _p not in sys.path:
        sys.path.insert(0, _p)

import ml_dtypes  # noqa: E402

import concourse.bass as bass  # noqa: E402
import concourse.tile as tile  # noqa: E402
from concourse import bacc, mybir  # noqa: E402
from concourse.bass_utils import run_bass_kernel_spmd  # noqa: E402


def _install_ntff_hook_shim():
    """antenv.axon_hooks is missing in this image, so the boot-time NTFF
    profile hook registration silently degraded.  Recreate the module and
    register the ctypes hook so trace=True yields exec_time_ns."""
    try:
        import antenv.axon_hooks  # noqa: F401
        return
    except ImportError:
        pass
    try:
        import types

        import antenv
        mod = types.ModuleType("antenv.axon_hooks")
        holder = {}
        mod.set_axon_ntff_profile_hook = lambda h: holder.__setitem__("h", h)
        mod.get_axon_ntff_profile_hook = lambda: holder.get("h")
        sys.modules["antenv.axon_hooks"] = mod
        antenv.axon_hooks = mod
        from trn_agent_boot.trn_boot import _ntff_profile_via_ctypes
        hook = _ntff_profile_via_ctypes("/opt/axon/libaxon_pjrt.so")
        if hook is not None:
            mod.set_axon_ntff_profile_hook(hook)
    except Exception:
        pass


_install_ntff_hook_shim()

BF16 = mybir.dt.bfloat16
F32 = mybir.dt.float32
NPBF16 = ml_dtypes.bfloat16

P = 128          # partitions / positions per tile
NT = 16          # seq tiles (2048 / 128)
NCHUNK = 4       # scalar-batching chunks (4 tiles each)
SEQ = 2048
DMODEL = 512
NCORES = 8

# zT path: "pe" = TensorE transpose (+psum->sbuf copy), "dma" = HWDGE xbar
ZT_PATH = os.environ.get("KERNEL_ZT_PATH", "pe")


def _build_nc():
    nc = bacc.Bacc("TRN2", target_bir_lowering=False, debug=False,
                   num_devices=NCORES)

    # xq[q, pd, j, c] = x^T[128*j + pd, 512*q + c] — one 512KB DMA per
    # quarter with 4KB-contiguous per-partition runs (full DMA rate)
    xT = nc.dram_tensor("xT", [4, P, 4, DMODEL], BF16,
                        kind="ExternalInput").ap()
    wqkv = nc.dram_tensor("wqkv", [P, 4, 384], BF16, kind="ExternalInput").ap()
    wo = nc.dram_tensor("wo", [P, DMODEL], BF16, kind="ExternalInput").ap()
    trit = nc.dram_tensor("trit", [P, P], BF16, kind="ExternalInput").ap()
    ident = nc.dram_tensor("ident", [P, P], BF16, kind="ExternalInput").ap()
    cnt = nc.dram_tensor("cnt", [P, NT, 2], F32, kind="ExternalInput").ap()
    # p-major output: out[p, t, d] = row 128*t+p of the logical [2048, 512]
    # partial (keeps the store DMA's per-partition runs 4KB-contiguous)
    out = nc.dram_tensor("out", [P, NT, DMODEL], BF16,
                         kind="ExternalOutput").ap()

    with tile.TileContext(nc) as tc:
        _body(tc, out, xT, wqkv, wo, trit, ident, cnt)
    nc.compile()
    return nc


def _body(tc, out, xT, wqkv, wo, trit, ident, cnt):
    nc = tc.nc
    mult = mybir.AluOpType.mult
    add = mybir.AluOpType.add
    Exp = mybir.ActivationFunctionType.Exp
    Copy = mybir.ActivationFunctionType.Copy

    with (
        tc.tile_pool(name="const", bufs=1) as const,
        tc.tile_pool(name="xpool", bufs=1) as xpool,
        tc.tile_pool(name="kvpool", bufs=3) as kvpool,
        tc.tile_pool(name="rpool", bufs=6) as rpool,
        tc.tile_pool(name="spool", bufs=1) as spool,
        tc.tile_pool(name="cscr", bufs=2) as cscr,
        tc.tile_pool(name="t1pool", bufs=3) as t1pool,
        tc.tile_pool(name="zpool", bufs=3) as zpool,
        tc.tile_pool(name="ztpool", bufs=3) as ztpool,
        tc.tile_pool(name="dscr", bufs=3) as dscr,
        tc.tile_pool(name="osb", bufs=3) as osb,
        tc.tile_pool(name="pproj", bufs=2, space="PSUM") as pproj,
        tc.tile_pool(name="psfx", bufs=2, space="PSUM") as psfx,
        tc.tile_pool(name="pout", bufs=2, space="PSUM") as pout,
    ):
        # ---- PE warm-up: ~5us of dummy matmuls during the DMA head so the
        # HAM clock gate reaches 2.4 GHz before real matmuls arrive ----
        wup = const.tile([P, DMODEL], BF16, name="wup")
        nc.vector.memset(wup[:], 0.0)
        pwup = pout.tile([P, DMODEL], F32, tag="po", name="pwup")
        for _ in range(12):
            nc.tensor.matmul(pwup[:], wup[:, 0:P], wup[:], start=True,
                             stop=True)

        # ---- weights (needed by first matmul), then x quarter-by-quarter
        # backward (processing runs backward from tile 15) ----
        wsb = const.tile([P, 4, 384], BF16)
        nc.sync.dma_start(wsb[:], wqkv[:])

        # xsb[pd, q, j, c]: quarter q, d-chunk j, col c (within quarter)
        xsb = xpool.tile([P, 4, 4, DMODEL], BF16)
        nc.sync.dma_start(xsb[:, 3, :, :], xT[3])
        nc.scalar.dma_start(xsb[:, 2, :, :], xT[2])

        trisb = const.tile([P, P], BF16)
        nc.gpsimd.dma_start(trisb[:], trit[:])
        cntsb = const.tile([P, NT, 2], F32)
        nc.gpsimd.dma_start(cntsb[:], cnt[:])
        wosb = const.tile([P, DMODEL], BF16)
        nc.gpsimd.dma_start(wosb[:], wo[:])
        onesb = const.tile([P, P], BF16)
        nc.vector.memset(onesb[:], 1.0)

        nc.sync.dma_start(xsb[:, 1, :, :], xT[1])
        nc.scalar.dma_start(xsb[:, 0, :, :], xT[0])

        # persistent per-position scalars: [128, tile(16), head(2)]
        s_both = spool.tile([P, NT, 2], F32)   # diag scores
        w_full = spool.tile([P, NT, 2], F32)   # 1/Z
        a_full = spool.tile([P, NT, 2], F32)   # (e-1)/Z

        R = {}     # R[t] = sum of v tiles t..15 (bf16)
        ztcs = {}
        v4s = {}

        # ---------- pipeline stages (emitted with 2-chunk rotation) ----------
        def stage_front(c):
            # proj at pair granularity into [128,2,512] psum tiles (2 banks,
            # double-buffered) so next pair's matmuls overlap this pair's
            # k/v evacuation; then diag + R chain + chunk scalars
            t0 = 4 * c
            v4 = kvpool.tile([P, 4, P], BF16, tag="v", name=f"v4_{c}")
            v4s[c] = v4
            for half in (1, 0):
                pq2 = pproj.tile([P, 2, DMODEL], F32, tag="pq",
                                 name=f"pq2_{c}_{half}")
                for dj in range(4):
                    for ii in (1, 0):
                        off = P * (2 * half + ii)
                        nc.tensor.matmul(
                            pq2[:, ii, 0:384],
                            xsb[:, c, dj, off:off + P],
                            wsb[:, dj, :],
                            start=(dj == 0),
                            stop=(dj == 3),
                        )
                k2 = kvpool.tile([P, 2, P], F32, tag="k",
                                 name=f"k2_{c}_{half}")
                nc.scalar.copy(k2[:], pq2[:, :, 128:256])
                nc.scalar.copy(v4[:, 2 * half:2 * half + 2, :],
                               pq2[:, :, 256:384])
                qk2 = dscr.tile([P, 2, 2, 64], F32, tag="qk",
                                name=f"qk2_{c}_{half}")
                nc.vector.tensor_mul(qk2[:], pq2[:, :, 0:128], k2[:])
                nc.vector.tensor_reduce(
                    s_both[:, t0 + 2 * half:t0 + 2 * half + 2, :], qk2[:],
                    axis=mybir.AxisListType.X, op=add)
                for t in reversed(range(t0 + 2 * half, t0 + 2 * half + 2)):
                    i = t - t0
                    if t == NT - 1:
                        R[t] = v4[:, i, :]
                    else:
                        r_new = rpool.tile([P, P], BF16, tag="r",
                                           name=f"r_{t}")
                        nc.vector.tensor_add(r_new[:], v4[:, i, :], R[t + 1])
                        R[t] = r_new[:]
            e_scr = cscr.tile([P, 4, 2], F32, tag="e", name=f"e_{c}")
            nc.scalar.activation(e_scr[:], s_both[:, t0:t0 + 4, :], Exp,
                                 scale=0.125)
            z_scr = cscr.tile([P, 4, 2], F32, tag="zz", name=f"zz_{c}")
            nc.vector.tensor_add(z_scr[:], e_scr[:], cntsb[:, t0:t0 + 4, :])
            nc.vector.reciprocal(w_full[:, t0:t0 + 4, :], z_scr[:])
            em1 = cscr.tile([P, 4, 2], F32, tag="em1", name=f"em1_{c}")
            nc.vector.tensor_scalar_add(em1[:], e_scr[:], -1.0)
            nc.vector.tensor_mul(a_full[:, t0:t0 + 4, :], em1[:],
                                 w_full[:, t0:t0 + 4, :])

        def stage_mid(c):
            # suffix matmuls, t1, fused combine, batched zT transpose
            t0 = 4 * c
            v4 = v4s[c]
            pss = {}
            for t in reversed(range(t0, t0 + 4)):
                i = t - t0
                ps = psfx.tile([P, P], F32, tag="ps", name=f"ps_{t}")
                nc.tensor.matmul(ps[:], trisb[:], v4[:, i, :],
                                 start=True, stop=(t == NT - 1))
                if t < NT - 1:
                    nc.tensor.matmul(ps[:], onesb[:], R[t + 1],
                                     start=False, stop=True)
                pss[t] = ps
            t1c = t1pool.tile([P, 4, P], F32, tag="t1", name=f"t1_{c}")
            for h in (0, 1):
                in0 = v4[:, :, 64 * h:64 * (h + 1)]
                in1 = a_full[:, t0:t0 + 4, h:h + 1]
                b0, b1 = bass.broadcast_tensor_aps(in0, in1)
                nc.vector.tensor_tensor(
                    t1c[:, :, 64 * h:64 * (h + 1)], b0, b1, op=mult)
            zc = zpool.tile([P, 4, P], BF16, tag="zc", name=f"zc_{c}")
            for t in reversed(range(t0, t0 + 4)):
                i = t - t0
                for h in (0, 1):
                    sl = slice(64 * h, 64 * (h + 1))
                    nc.vector.scalar_tensor_tensor(
                        zc[:, i, sl], pss[t][:, sl],
                        w_full[:, t, h:h + 1], t1c[:, i, sl],
                        op0=mult, op1=add)
            ztc = ztpool.tile([P, 4, P], BF16, tag="ztc", name=f"ztc_{c}")
            nc.sync.dma_start_transpose(ztc[:], zc[:])
            ztcs[c] = ztc

        # rotation: M(c+1) | F(c) | B(c+2).  M first so its matmuls are not
        # queued behind F's x-DMA-gated matmuls; B last so its W_O matmuls
        # consume a transpose issued a full iteration earlier (keeps PE from
        # idling >3.4us, which would re-throttle the HAM clock gate)
        for c in reversed(range(NCHUNK)):
            if c + 1 < NCHUNK:
                stage_mid(c + 1)
            stage_front(c)
            if c + 2 < NCHUNK:
                _emit_out_stage(nc, c + 2, ztcs[c + 2], pout, osb, wosb, out)
        stage_mid(0)
        _emit_out_stage(nc, 1, ztcs[1], pout, osb, wosb, out)
        _emit_out_stage(nc, 0, ztcs[0], pout, osb, wosb, out)


def _emit_out_stage(nc, c, ztc, pout, osb, wosb, out):
    o4 = osb.tile([P, 4, DMODEL], BF16, tag="o", name=f"o4_{c}")
    for t in reversed(range(4 * c, 4 * c + 4)):
        i = t - 4 * c
        po = pout.tile([P, DMODEL], F32, tag="po", name=f"po_{t}")
        nc.tensor.matmul(po[:], ztc[:, i, :], wosb[:], start=True, stop=True)
        nc.vector.tensor_copy(o4[:, i, 0:192], po[:, 0:192])
        nc.scalar.copy(o4[:, i, 192:512], po[:, 192:512])
    # one batched DMA for the whole chunk (p-major dram layout)
    t0 = 4 * c
    nc.sync.dma_start(out[:, t0:t0 + 4, :], o4[:])


_NC_CACHE = {}


def _get_nc():
    if "nc" not in _NC_CACHE:
        _NC_CACHE["nc"] = _build_nc()
    return _NC_CACHE["nc"]


def _make_in_maps(x, W_Q, W_K, W_V, W_O):
    tri = np.tril(np.ones((P, P), np.float32)).astype(NPBF16)
    ident = np.eye(P, dtype=np.float32).astype(NPBF16)
    pos = (np.arange(NT)[None, :] * P + np.arange(P)[:, None]).astype(np.float32)
    cnt1 = (SEQ - 1) - pos                      # [128, 16]
    cnt = np.stack([cnt1, cnt1], axis=2)        # [128, 16, 2]
    cnt = np.ascontiguousarray(cnt, np.float32)

    in_maps = []
    for core in range(NCORES):
        b, g = core // 4, core % 4
        xTb = np.asarray(x[b]).T.astype(NPBF16)          # [512, 2048]
        # -> [q, pd, j, c]: xq[q, pd, j, c] = xT[128j+pd, 512q+c]
        xq = np.ascontiguousarray(
            xTb.reshape(4, P, 4, DMODEL).transpose(2, 1, 0, 3))
        wq = np.asarray(W_Q[2 * g:2 * g + 2]).reshape(P, DMODEL).T
        wk = np.asarray(W_K[2 * g:2 * g + 2]).reshape(P, DMODEL).T
        wv = np.asarray(W_V[2 * g:2 * g + 2]).reshape(P, DMODEL).T
        wqkv = np.concatenate([wq, wk, wv], axis=1).astype(NPBF16)  # [512,384]
        # -> [pd, j, 384]: whost[pd, j, :] = wqkv[128j+pd, :]
        whost = np.ascontiguousarray(
            wqkv.reshape(4, P, 384).transpose(1, 0, 2))
        wo_c = np.ascontiguousarray(
            np.asarray(W_O)[:, P * g:P * (g + 1)].T).astype(NPBF16)
        in_maps.append({
            "xT": xq, "wqkv": whost, "wo": wo_c,
            "trit": tri, "ident": ident, "cnt": cnt,
        })
    return in_maps


def _run(x, W_Q, W_K, W_V, W_O, trace=False, **spmd_kwargs):
    nc = _get_nc()
    in_maps = _make_in_maps(x, W_Q, W_K, W_V, W_O)
    res = run_bass_kernel_spmd(nc, in_maps, core_ids=list(range(NCORES)),
                               trace=trace, **spmd_kwargs)
    # device output is p-major [128, 16, 512]; back to [2048, 512]
    outs = [r["out"].astype(np.float32).transpose(1, 0, 2).reshape(SEQ, DMODEL)
            for r in res.results]
    full = np.stack([
        outs[0] + outs[1] + outs[2] + outs[3],
        outs[4] + outs[5] + outs[6] + outs[7],
    ])  # [2, 2048, 512]
    return full, res


def kernel(x, W_Q, W_K, W_V, W_O):
    full, _ = _run(np.asarray(x), np.asarray(W_Q), np.asarray(W_K),
                   np.asarray(W_V), np.asarray(W_O))
    return full


# revision 63
# speedup vs baseline: 1.1130x; 1.1130x over previous
"""Trainium2 Bass kernel for nn_Attention_63995012710903.

Math: the reference's mask is `scores*tril - 1e10*(1-triu)`, which makes the
softmax argument (pre /sqrt(64)):
    p <  q : scores - 1e10   -> exp underflows to exactly 0 in fp32
    p == q : scores
    p >  q : 0               -> exp = 1
So attention collapses to, per (batch, head), with e_q = exp(s_qq/8),
cnt_q = n-1-q, Z_q = e_q + cnt_q:
    z[q,:] = (e_q * v[q,:] + sum_{p>q} v[p,:]) / Z_q
           = ((e_q - 1) * v[q,:] + S_incl[q,:]) / Z_q,   S_incl[q] = sum_{p>=q} v[p]
Only the DIAGONAL of the score matrix and suffix sums of V are needed; both
O(n^2) attention matmuls disappear.  out = z_flat @ W_O^T.

Sharding (8 cores): core c -> batch b=c//4, head-pair g=c%4 (heads 2g, 2g+1).
Each core computes q,k,v for its 2 heads (f-block of 128 of z_flat's 512
features), z for that block, and the PARTIAL output  z_blk @ W_O[:,blk]^T
([2048, 512]).  The unshard sums the 4 partials per batch on host (W_O is
column-sharded, per the problem's sharding hint).

Device layout: positions on partitions ("p-layout").  x^T ([512,2048]) is
prepared host-side so the d=512 contraction sits on partitions.  bf16 compute
(PSUM accumulates fp32); suffix sums via matmuls with a lower-triangular
ones matrix plus a running tile-sum R.
"""

import os
import sys

import numpy as np

for _p in ("/opt/trn_rl_repo", "/root/.axon_site/_ro/trn_rl_repo"):
    if os.path.isdir(_p) and _p not in sys.path:
        sys.path.insert(0, _p)

import ml_dtypes  # noqa: E402

import concourse.bass as bass  # noqa: E402
import concourse.tile as tile  # noqa: E402
from concourse import bacc, mybir  # noqa: E402
from concourse.bass_utils import run_bass_kernel_spmd  # noqa: E402


def _install_ntff_hook_shim():
    """antenv.axon_hooks is missing in this image, so the boot-time NTFF
    profile hook registration silently degraded.  Recreate the module and
    register the ctypes hook so trace=True yields exec_time_ns."""
    try:
        import antenv.axon_hooks  # noqa: F401
        return
    except ImportError:
        pass
    try:
        import types

        import antenv
        mod = types.ModuleType("antenv.axon_hooks")
        holder = {}
        mod.set_axon_ntff_profile_hook = lambda h: holder.__setitem__("h", h)
        mod.get_axon_ntff_profile_hook = lambda: holder.get("h")
        sys.modules["antenv.axon_hooks"] = mod
        antenv.axon_hooks = mod
        from trn_agent_boot.trn_boot import _ntff_profile_via_ctypes
        hook = _ntff_profile_via_ctypes("/opt/axon/libaxon_pjrt.so")
        if hook is not None:
            mod.set_axon_ntff_profile_hook(hook)
    except Exception:
        pass


_install_ntff_hook_shim()

BF16 = mybir.dt.bfloat16
F32 = mybir.dt.float32
NPBF16 = ml_dtypes.bfloat16

P = 128          # partitions / positions per tile
NT = 16          # seq tiles (2048 / 128)
NCHUNK = 4       # scalar-batching chunks (4 tiles each)
SEQ = 2048
DMODEL = 512
NCORES = 8

# zT path: "pe" = TensorE transpose (+psum->sbuf copy), "dma" = HWDGE xbar
ZT_PATH = os.environ.get("KERNEL_ZT_PATH", "pe")


def _build_nc():
    nc = bacc.Bacc("TRN2", target_bir_lowering=False, debug=False,
                   num_devices=NCORES)

    # xq[q, pd, j, c] = x^T[128*j + pd, 512*q + c] — one 512KB DMA per
    # quarter with 4KB-contiguous per-partition runs (full DMA rate)
    xT = nc.dram_tensor("xT", [4, P, 4, DMODEL], BF16,
                        kind="ExternalInput").ap()
    wqkv = nc.dram_tensor("wqkv", [P, 4, 384], BF16, kind="ExternalInput").ap()
    wo = nc.dram_tensor("wo", [P, DMODEL], BF16, kind="ExternalInput").ap()
    trit = nc.dram_tensor("trit", [P, P], BF16, kind="ExternalInput").ap()
    ident = nc.dram_tensor("ident", [P, P], BF16, kind="ExternalInput").ap()
    cnt = nc.dram_tensor("cnt", [P, NT, 2], F32, kind="ExternalInput").ap()
    # p-major output: out[p, t, d] = row 128*t+p of the logical [2048, 512]
    # partial (keeps the store DMA's per-partition runs 4KB-contiguous)
    out = nc.dram_tensor("out", [P, NT, DMODEL], BF16,
                         kind="ExternalOutput").ap()

    with tile.TileContext(nc) as tc:
        _body(tc, out, xT, wqkv, wo, trit, ident, cnt)
    nc.compile()
    return nc


def _body(tc, out, xT, wqkv, wo, trit, ident, cnt):
    nc = tc.nc
    mult = mybir.AluOpType.mult
    add = mybir.AluOpType.add
    Exp = mybir.ActivationFunctionType.Exp
    Copy = mybir.ActivationFunctionType.Copy

    with (
        tc.tile_pool(name="const", bufs=1) as const,
        tc.tile_pool(name="xpool", bufs=1) as xpool,
        tc.tile_pool(name="kvpool", bufs=3) as kvpool,
        tc.tile_pool(name="rpool", bufs=6) as rpool,
        tc.tile_pool(name="spool", bufs=1) as spool,
        tc.tile_pool(name="cscr", bufs=2) as cscr,
        tc.tile_pool(name="t1pool", bufs=3) as t1pool,
        tc.tile_pool(name="zpool", bufs=3) as zpool,
        tc.tile_pool(name="ztpool", bufs=3) as ztpool,
        tc.tile_pool(name="dscr", bufs=3) as dscr,
        tc.tile_pool(name="osb", bufs=3) as osb,
        tc.tile_pool(name="pproj", bufs=2, space="PSUM") as pproj,
        tc.tile_pool(name="psfx", bufs=2, space="PSUM") as psfx,
        tc.tile_pool(name="pout", bufs=2, space="PSUM") as pout,
    ):
        # ---- PE warm-up: ~5us of dummy matmuls during the DMA head so the
        # HAM clock gate reaches 2.4 GHz before real matmuls arrive ----
        wup = const.tile([P, DMODEL], BF16, name="wup")
        nc.vector.memset(wup[:], 0.0)
        pwup = pout.tile([P, DMODEL], F32, tag="po", name="pwup")
        for _ in range(12):
            nc.tensor.matmul(pwup[:], wup[:, 0:P], wup[:], start=True,
                             stop=True)

        # ---- weights (needed by first matmul), then x quarter-by-quarter
        # backward (processing runs backward from tile 15) ----
        wsb = const.tile([P, 4, 384], BF16)
        nc.sync.dma_start(wsb[:], wqkv[:])

        # xsb[pd, q, j, c]: quarter q, d-chunk j, col c (within quarter)
        xsb = xpool.tile([P, 4, 4, DMODEL], BF16)
        nc.sync.dma_start(xsb[:, 3, :, :], xT[3])

        trisb = const.tile([P, P], BF16)
        nc.gpsimd.dma_start(trisb[:], trit[:])
        cntsb = const.tile([P, NT, 2], F32)
        nc.gpsimd.dma_start(cntsb[:], cnt[:])
        wosb = const.tile([P, DMODEL], BF16)
        nc.gpsimd.dma_start(wosb[:], wo[:])
        onesb = const.tile([P, P], BF16)
        nc.vector.memset(onesb[:], 1.0)

        for q in (2, 1, 0):
            nc.sync.dma_start(xsb[:, q, :, :], xT[q])

        # persistent per-position scalars: [128, tile(16), head(2)]
        s_both = spool.tile([P, NT, 2], F32)   # diag scores
        w_full = spool.tile([P, NT, 2], F32)   # 1/Z
        a_full = spool.tile([P, NT, 2], F32)   # (e-1)/Z

        R = {}     # R[t] = sum of v tiles t..15 (bf16)
        ztcs = {}
        v4s = {}

        # ---------- pipeline stages (emitted with 2-chunk rotation) ----------
        def stage_front(c):
            # proj at pair granularity into [128,2,512] psum tiles (2 banks,
            # double-buffered) so next pair's matmuls overlap this pair's
            # k/v evacuation; then diag + R chain + chunk scalars
            t0 = 4 * c
            v4 = kvpool.tile([P, 4, P], BF16, tag="v", name=f"v4_{c}")
            v4s[c] = v4
            for half in (1, 0):
                pq2 = pproj.tile([P, 2, DMODEL], F32, tag="pq",
                                 name=f"pq2_{c}_{half}")
                for dj in range(4):
                    for ii in (1, 0):
                        off = P * (2 * half + ii)
                        nc.tensor.matmul(
                            pq2[:, ii, 0:384],
                            xsb[:, c, dj, off:off + P],
                            wsb[:, dj, :],
                            start=(dj == 0),
                            stop=(dj == 3),
                        )
                k2 = kvpool.tile([P, 2, P], F32, tag="k",
                                 name=f"k2_{c}_{half}")
                nc.scalar.copy(k2[:], pq2[:, :, 128:256])
                nc.scalar.copy(v4[:, 2 * half:2 * half + 2, :],
                               pq2[:, :, 256:384])
                qk2 = dscr.tile([P, 2, 2, 64], F32, tag="qk",
                                name=f"qk2_{c}_{half}")
                nc.vector.tensor_mul(qk2[:], pq2[:, :, 0:128], k2[:])
                nc.vector.tensor_reduce(
                    s_both[:, t0 + 2 * half:t0 + 2 * half + 2, :], qk2[:],
                    axis=mybir.AxisListType.X, op=add)
                for t in reversed(range(t0 + 2 * half, t0 + 2 * half + 2)):
                    i = t - t0
                    if t == NT - 1:
                        R[t] = v4[:, i, :]
                    else:
                        r_new = rpool.tile([P, P], BF16, tag="r",
                                           name=f"r_{t}")
                        nc.vector.tensor_add(r_new[:], v4[:, i, :], R[t + 1])
                        R[t] = r_new[:]
            e_scr = cscr.tile([P, 4, 2], F32, tag="e", name=f"e_{c}")
            nc.scalar.activation(e_scr[:], s_both[:, t0:t0 + 4, :], Exp,
                                 scale=0.125)
            z_scr = cscr.tile([P, 4, 2], F32, tag="zz", name=f"zz_{c}")
            nc.vector.tensor_add(z_scr[:], e_scr[:], cntsb[:, t0:t0 + 4, :])
            nc.vector.reciprocal(w_full[:, t0:t0 + 4, :], z_scr[:])
            em1 = cscr.tile([P, 4, 2], F32, tag="em1", name=f"em1_{c}")
            nc.vector.tensor_scalar_add(em1[:], e_scr[:], -1.0)
            nc.vector.tensor_mul(a_full[:, t0:t0 + 4, :], em1[:],
                                 w_full[:, t0:t0 + 4, :])

        def stage_mid(c):
            # suffix matmuls, t1, fused combine, batched zT transpose
            t0 = 4 * c
            v4 = v4s[c]
            pss = {}
            for t in reversed(range(t0, t0 + 4)):
                i = t - t0
                ps = psfx.tile([P, P], F32, tag="ps", name=f"ps_{t}")
                nc.tensor.matmul(ps[:], trisb[:], v4[:, i, :],
                                 start=True, stop=(t == NT - 1))
                if t < NT - 1:
                    nc.tensor.matmul(ps[:], onesb[:], R[t + 1],
                                     start=False, stop=True)
                pss[t] = ps
            t1c = t1pool.tile([P, 4, P], F32, tag="t1", name=f"t1_{c}")
            for h in (0, 1):
                in0 = v4[:, :, 64 * h:64 * (h + 1)]
                in1 = a_full[:, t0:t0 + 4, h:h + 1]
                b0, b1 = bass.broadcast_tensor_aps(in0, in1)
                nc.vector.tensor_tensor(
                    t1c[:, :, 64 * h:64 * (h + 1)], b0, b1, op=mult)
            zc = zpool.tile([P, 4, P], BF16, tag="zc", name=f"zc_{c}")
            for t in reversed(range(t0, t0 + 4)):
                i = t - t0
                for h in (0, 1):
                    sl = slice(64 * h, 64 * (h + 1))
                    nc.vector.scalar_tensor_tensor(
                        zc[:, i, sl], pss[t][:, sl],
                        w_full[:, t, h:h + 1], t1c[:, i, sl],
                        op0=mult, op1=add)
            ztc = ztpool.tile([P, 4, P], BF16, tag="ztc", name=f"ztc_{c}")
            nc.sync.dma_start_transpose(ztc[:], zc[:])
            ztcs[c] = ztc

        # rotation: M(c+1) | F(c) | B(c+2).  M first so its matmuls are not
        # queued behind F's x-DMA-gated matmuls; B last so its W_O matmuls
        # consume a transpose issued a full iteration earlier (keeps PE from
        # idling >3.4us, which would re-throttle the HAM clock gate)
        for c in reversed(range(NCHUNK)):
            if c + 1 < NCHUNK:
                stage_mid(c + 1)
            stage_front(c)
            if c + 2 < NCHUNK:
                _emit_out_stage(nc, c + 2, ztcs[c + 2], pout, osb, wosb, out)
        _emit_out_stage(nc, 1, ztcs[1], pout, osb, wosb, out)

        # ---- epilogue: final chunk with pair-pipelined transpose/W_O so
        # the tail chain is [stt hi | T hi | stt lo + WO hi | T lo | WO lo]
        v4 = v4s[0]
        pss = {}
        for t in (3, 2, 1, 0):
            ps = psfx.tile([P, P], F32, tag="ps", name=f"ps_{t}")
            nc.tensor.matmul(ps[:], trisb[:], v4[:, t, :],
                             start=True, stop=False)
            nc.tensor.matmul(ps[:], onesb[:], R[t + 1],
                             start=False, stop=True)
            pss[t] = ps
        t1c = t1pool.tile([P, 4, P], F32, tag="t1", name="t1_ep")
        for h in (0, 1):
            b0, b1 = bass.broadcast_tensor_aps(
                v4[:, :, 64 * h:64 * (h + 1)], a_full[:, 0:4, h:h + 1])
            nc.vector.tensor_tensor(
                t1c[:, :, 64 * h:64 * (h + 1)], b0, b1, op=mult)
        zc = zpool.tile([P, 4, P], BF16, tag="zc", name="zc_ep")
        ztc_pairs = {}
        o4 = osb.tile([P, 4, DMODEL], BF16, tag="o", name="o4_ep")
        for half in (1, 0):
            for i in (2 * half + 1, 2 * half):
                for h in (0, 1):
                    sl = slice(64 * h, 64 * (h + 1))
                    nc.vector.scalar_tensor_tensor(
                        zc[:, i, sl], pss[i][:, sl],
                        w_full[:, i, h:h + 1], t1c[:, i, sl],
                        op0=mult, op1=add)
            ztc2 = ztpool.tile([P, 2, P], BF16, tag="ztc2",
                               name=f"ztc_ep_{half}")
            nc.sync.dma_start_transpose(ztc2[:], zc[:, 2 * half:2 * half + 2, :])
            ztc_pairs[half] = ztc2
            for i in (2 * half + 1, 2 * half):
                po = pout.tile([P, DMODEL], F32, tag="po", name=f"po_ep{i}")
                nc.tensor.matmul(po[:], ztc2[:, i % 2, :], wosb[:],
                                 start=True, stop=True)
                nc.vector.tensor_copy(o4[:, i, 0:192], po[:, 0:192])
                nc.scalar.copy(o4[:, i, 192:512], po[:, 192:512])
            nc.sync.dma_start(
                out[:, 2 * half:2 * half + 2, :],
                o4[:, 2 * half:2 * half + 2, :])


def _emit_out_stage(nc, c, ztc, pout, osb, wosb, out):
    o4 = osb.tile([P, 4, DMODEL], BF16, tag="o", name=f"o4_{c}")
    for t in reversed(range(4 * c, 4 * c + 4)):
        i = t - 4 * c
        po = pout.tile([P, DMODEL], F32, tag="po", name=f"po_{t}")
        nc.tensor.matmul(po[:], ztc[:, i, :], wosb[:], start=True, stop=True)
        nc.vector.tensor_copy(o4[:, i, 0:192], po[:, 0:192])
        nc.scalar.copy(o4[:, i, 192:512], po[:, 192:512])
    # one batched DMA for the whole chunk (p-major dram layout)
    t0 = 4 * c
    nc.sync.dma_start(out[:, t0:t0 + 4, :], o4[:])


_NC_CACHE = {}


def _get_nc():
    if "nc" not in _NC_CACHE:
        _NC_CACHE["nc"] = _build_nc()
    return _NC_CACHE["nc"]


def _make_in_maps(x, W_Q, W_K, W_V, W_O):
    tri = np.tril(np.ones((P, P), np.float32)).astype(NPBF16)
    ident = np.eye(P, dtype=np.float32).astype(NPBF16)
    pos = (np.arange(NT)[None, :] * P + np.arange(P)[:, None]).astype(np.float32)
    cnt1 = (SEQ - 1) - pos                      # [128, 16]
    cnt = np.stack([cnt1, cnt1], axis=2)        # [128, 16, 2]
    cnt = np.ascontiguousarray(cnt, np.float32)

    in_maps = []
    for core in range(NCORES):
        b, g = core // 4, core % 4
        xTb = np.asarray(x[b]).T.astype(NPBF16)          # [512, 2048]
        # -> [q, pd, j, c]: xq[q, pd, j, c] = xT[128j+pd, 512q+c]
        xq = np.ascontiguousarray(
            xTb.reshape(4, P, 4, DMODEL).transpose(2, 1, 0, 3))
        wq = np.asarray(W_Q[2 * g:2 * g + 2]).reshape(P, DMODEL).T
        wk = np.asarray(W_K[2 * g:2 * g + 2]).reshape(P, DMODEL).T
        wv = np.asarray(W_V[2 * g:2 * g + 2]).reshape(P, DMODEL).T
        wqkv = np.concatenate([wq, wk, wv], axis=1).astype(NPBF16)  # [512,384]
        # -> [pd, j, 384]: whost[pd, j, :] = wqkv[128j+pd, :]
        whost = np.ascontiguousarray(
            wqkv.reshape(4, P, 384).transpose(1, 0, 2))
        wo_c = np.ascontiguousarray(
            np.asarray(W_O)[:, P * g:P * (g + 1)].T).astype(NPBF16)
        in_maps.append({
            "xT": xq, "wqkv": whost, "wo": wo_c,
            "trit": tri, "ident": ident, "cnt": cnt,
        })
    return in_maps


def _run(x, W_Q, W_K, W_V, W_O, trace=False, **spmd_kwargs):
    nc = _get_nc()
    in_maps = _make_in_maps(x, W_Q, W_K, W_V, W_O)
    res = run_bass_kernel_spmd(nc, in_maps, core_ids=list(range(NCORES)),
                               trace=trace, **spmd_kwargs)
    # device output is p-major [128, 16, 512]; back to [2048, 512]
    outs = [r["out"].astype(np.float32).transpose(1, 0, 2).reshape(SEQ, DMODEL)
            for r in res.results]
    full = np.stack([
        outs[0] + outs[1] + outs[2] + outs[3],
        outs[4] + outs[5] + outs[6] + outs[7],
    ])  # [2, 2048, 512]
    return full, res


def kernel(x, W_Q, W_K, W_V, W_O):
    full, _ = _run(np.asarray(x), np.asarray(W_Q), np.asarray(W_K),
                   np.asarray(W_V), np.asarray(W_O))
    return full


# revision 64
# speedup vs baseline: 1.1143x; 1.0011x over previous
"""Trainium2 Bass kernel for nn_Attention_63995012710903.

Math: the reference's mask is `scores*tril - 1e10*(1-triu)`, which makes the
softmax argument (pre /sqrt(64)):
    p <  q : scores - 1e10   -> exp underflows to exactly 0 in fp32
    p == q : scores
    p >  q : 0               -> exp = 1
So attention collapses to, per (batch, head), with e_q = exp(s_qq/8),
cnt_q = n-1-q, Z_q = e_q + cnt_q:
    z[q,:] = (e_q * v[q,:] + sum_{p>q} v[p,:]) / Z_q
           = ((e_q - 1) * v[q,:] + S_incl[q,:]) / Z_q,   S_incl[q] = sum_{p>=q} v[p]
Only the DIAGONAL of the score matrix and suffix sums of V are needed; both
O(n^2) attention matmuls disappear.  out = z_flat @ W_O^T.

Sharding (8 cores): core c -> batch b=c//4, head-pair g=c%4 (heads 2g, 2g+1).
Each core computes q,k,v for its 2 heads (f-block of 128 of z_flat's 512
features), z for that block, and the PARTIAL output  z_blk @ W_O[:,blk]^T
([2048, 512]).  The unshard sums the 4 partials per batch on host (W_O is
column-sharded, per the problem's sharding hint).

Device layout: positions on partitions ("p-layout").  x^T ([512,2048]) is
prepared host-side so the d=512 contraction sits on partitions.  bf16 compute
(PSUM accumulates fp32); suffix sums via matmuls with a lower-triangular
ones matrix plus a running tile-sum R.
"""

import os
import sys

import numpy as np

for _p in ("/opt/trn_rl_repo", "/root/.axon_site/_ro/trn_rl_repo"):
    if os.path.isdir(_p) and _p not in sys.path:
        sys.path.insert(0, _p)

import ml_dtypes  # noqa: E402

import concourse.bass as bass  # noqa: E402
import concourse.tile as tile  # noqa: E402
from concourse import bacc, mybir  # noqa: E402
from concourse.bass_utils import run_bass_kernel_spmd  # noqa: E402


def _install_ntff_hook_shim():
    """antenv.axon_hooks is missing in this image, so the boot-time NTFF
    profile hook registration silently degraded.  Recreate the module and
    register the ctypes hook so trace=True yields exec_time_ns."""
    try:
        import antenv.axon_hooks  # noqa: F401
        return
    except ImportError:
        pass
    try:
        import types

        import antenv
        mod = types.ModuleType("antenv.axon_hooks")
        holder = {}
        mod.set_axon_ntff_profile_hook = lambda h: holder.__setitem__("h", h)
        mod.get_axon_ntff_profile_hook = lambda: holder.get("h")
        sys.modules["antenv.axon_hooks"] = mod
        antenv.axon_hooks = mod
        from trn_agent_boot.trn_boot import _ntff_profile_via_ctypes
        hook = _ntff_profile_via_ctypes("/opt/axon/libaxon_pjrt.so")
        if hook is not None:
            mod.set_axon_ntff_profile_hook(hook)
    except Exception:
        pass


_install_ntff_hook_shim()

BF16 = mybir.dt.bfloat16
F32 = mybir.dt.float32
NPBF16 = ml_dtypes.bfloat16

P = 128          # partitions / positions per tile
NT = 16          # seq tiles (2048 / 128)
NCHUNK = 4       # scalar-batching chunks (4 tiles each)
SEQ = 2048
DMODEL = 512
NCORES = 8

# zT path: "pe" = TensorE transpose (+psum->sbuf copy), "dma" = HWDGE xbar
ZT_PATH = os.environ.get("KERNEL_ZT_PATH", "pe")


def _build_nc():
    nc = bacc.Bacc("TRN2", target_bir_lowering=False, debug=False,
                   num_devices=NCORES)

    # xq[q, pd, j, c] = x^T[128*j + pd, 512*q + c] — one 512KB DMA per
    # quarter with 4KB-contiguous per-partition runs (full DMA rate)
    xT = nc.dram_tensor("xT", [4, P, 4, DMODEL], BF16,
                        kind="ExternalInput").ap()
    wqkv = nc.dram_tensor("wqkv", [P, 4, 384], BF16, kind="ExternalInput").ap()
    wo = nc.dram_tensor("wo", [P, DMODEL], BF16, kind="ExternalInput").ap()
    trit = nc.dram_tensor("trit", [P, P], BF16, kind="ExternalInput").ap()
    ident = nc.dram_tensor("ident", [P, P], BF16, kind="ExternalInput").ap()
    cnt = nc.dram_tensor("cnt", [P, NT, 2], F32, kind="ExternalInput").ap()
    # p-major output: out[p, t, d] = row 128*t+p of the logical [2048, 512]
    # partial (keeps the store DMA's per-partition runs 4KB-contiguous)
    out = nc.dram_tensor("out", [P, NT, DMODEL], BF16,
                         kind="ExternalOutput").ap()

    with tile.TileContext(nc) as tc:
        _body(tc, out, xT, wqkv, wo, trit, ident, cnt)
    nc.compile()
    return nc


def _body(tc, out, xT, wqkv, wo, trit, ident, cnt):
    nc = tc.nc
    mult = mybir.AluOpType.mult
    add = mybir.AluOpType.add
    Exp = mybir.ActivationFunctionType.Exp
    Copy = mybir.ActivationFunctionType.Copy

    with (
        tc.tile_pool(name="const", bufs=1) as const,
        tc.tile_pool(name="xpool", bufs=1) as xpool,
        tc.tile_pool(name="kvpool", bufs=3) as kvpool,
        tc.tile_pool(name="rpool", bufs=6) as rpool,
        tc.tile_pool(name="spool", bufs=1) as spool,
        tc.tile_pool(name="cscr", bufs=2) as cscr,
        tc.tile_pool(name="t1pool", bufs=3) as t1pool,
        tc.tile_pool(name="zpool", bufs=3) as zpool,
        tc.tile_pool(name="ztpool", bufs=3) as ztpool,
        tc.tile_pool(name="dscr", bufs=3) as dscr,
        tc.tile_pool(name="osb", bufs=3) as osb,
        tc.tile_pool(name="pproj", bufs=2, space="PSUM") as pproj,
        tc.tile_pool(name="psfx", bufs=2, space="PSUM") as psfx,
        tc.tile_pool(name="pout", bufs=2, space="PSUM") as pout,
    ):
        # ---- PE warm-up: ~5us of dummy matmuls during the DMA head so the
        # HAM clock gate reaches 2.4 GHz before real matmuls arrive ----
        wup = const.tile([P, DMODEL], BF16, name="wup")
        nc.vector.memset(wup[:], 0.0)
        pwup = pout.tile([P, DMODEL], F32, tag="po", name="pwup")
        for _ in range(12):
            nc.tensor.matmul(pwup[:], wup[:, 0:P], wup[:], start=True,
                             stop=True)

        # ---- weights (needed by first matmul), then x quarter-by-quarter
        # backward (processing runs backward from tile 15) ----
        wsb = const.tile([P, 4, 384], BF16)
        nc.sync.dma_start(wsb[:], wqkv[:])

        # xsb[pd, q, j, c]: quarter q, d-chunk j, col c (within quarter)
        xsb = xpool.tile([P, 4, 4, DMODEL], BF16)
        nc.sync.dma_start(xsb[:, 3, :, :], xT[3])

        trisb = const.tile([P, P], BF16)
        nc.gpsimd.dma_start(trisb[:], trit[:])
        cntsb = const.tile([P, NT, 2], F32)
        nc.gpsimd.dma_start(cntsb[:], cnt[:])
        wosb = const.tile([P, DMODEL], BF16)
        nc.gpsimd.dma_start(wosb[:], wo[:])
        onesb = const.tile([P, P], BF16)
        nc.vector.memset(onesb[:], 1.0)

        for q in (2, 1, 0):
            nc.sync.dma_start(xsb[:, q, :, :], xT[q])

        # persistent per-position scalars: [128, tile(16), head(2)]
        s_both = spool.tile([P, NT, 2], F32)   # diag scores
        w_full = spool.tile([P, NT, 2], F32)   # 1/Z
        a_full = spool.tile([P, NT, 2], F32)   # (e-1)/Z

        R = {}     # R[t] = sum of v tiles t..15 (bf16)
        ztcs = {}
        v4s = {}

        # ---------- pipeline stages (emitted with 2-chunk rotation) ----------
        def stage_front(c):
            # proj at pair granularity into [128,2,512] psum tiles (2 banks,
            # double-buffered) so next pair's matmuls overlap this pair's
            # k/v evacuation; then diag + R chain + chunk scalars
            t0 = 4 * c
            v4 = kvpool.tile([P, 4, P], BF16, tag="v", name=f"v4_{c}")
            v4s[c] = v4
            for half in (1, 0):
                pq2 = pproj.tile([P, 2, DMODEL], F32, tag="pq",
                                 name=f"pq2_{c}_{half}")
                for dj in range(4):
                    for ii in (1, 0):
                        off = P * (2 * half + ii)
                        nc.tensor.matmul(
                            pq2[:, ii, 0:384],
                            xsb[:, c, dj, off:off + P],
                            wsb[:, dj, :],
                            start=(dj == 0),
                            stop=(dj == 3),
                        )
                k2 = kvpool.tile([P, 2, P], F32, tag="k",
                                 name=f"k2_{c}_{half}")
                nc.scalar.copy(k2[:], pq2[:, :, 128:256])
                nc.scalar.copy(v4[:, 2 * half:2 * half + 2, :],
                               pq2[:, :, 256:384])
                qk2 = dscr.tile([P, 2, 2, 64], F32, tag="qk",
                                name=f"qk2_{c}_{half}")
                nc.vector.tensor_mul(qk2[:], pq2[:, :, 0:128], k2[:])
                nc.vector.tensor_reduce(
                    s_both[:, t0 + 2 * half:t0 + 2 * half + 2, :], qk2[:],
                    axis=mybir.AxisListType.X, op=add)
                for t in reversed(range(t0 + 2 * half, t0 + 2 * half + 2)):
                    i = t - t0
                    if t == NT - 1:
                        R[t] = v4[:, i, :]
                    else:
                        r_new = rpool.tile([P, P], BF16, tag="r",
                                           name=f"r_{t}")
                        nc.vector.tensor_add(r_new[:], v4[:, i, :], R[t + 1])
                        R[t] = r_new[:]
            e_scr = cscr.tile([P, 4, 2], F32, tag="e", name=f"e_{c}")
            nc.scalar.activation(e_scr[:], s_both[:, t0:t0 + 4, :], Exp,
                                 scale=0.125)
            z_scr = cscr.tile([P, 4, 2], F32, tag="zz", name=f"zz_{c}")
            nc.vector.tensor_add(z_scr[:], e_scr[:], cntsb[:, t0:t0 + 4, :])
            nc.vector.reciprocal(w_full[:, t0:t0 + 4, :], z_scr[:])
            em1 = cscr.tile([P, 4, 2], F32, tag="em1", name=f"em1_{c}")
            nc.vector.tensor_scalar_add(em1[:], e_scr[:], -1.0)
            nc.vector.tensor_mul(a_full[:, t0:t0 + 4, :], em1[:],
                                 w_full[:, t0:t0 + 4, :])

        def stage_mid(c):
            # suffix matmuls, t1, fused combine, batched zT transpose
            t0 = 4 * c
            v4 = v4s[c]
            pss = {}
            for t in reversed(range(t0, t0 + 4)):
                i = t - t0
                ps = psfx.tile([P, P], F32, tag="ps", name=f"ps_{t}")
                nc.tensor.matmul(ps[:], trisb[:], v4[:, i, :],
                                 start=True, stop=(t == NT - 1))
                if t < NT - 1:
                    nc.tensor.matmul(ps[:], onesb[:], R[t + 1],
                                     start=False, stop=True)
                pss[t] = ps
            t1c = t1pool.tile([P, 4, P], F32, tag="t1", name=f"t1_{c}")
            for h in (0, 1):
                in0 = v4[:, :, 64 * h:64 * (h + 1)]
                in1 = a_full[:, t0:t0 + 4, h:h + 1]
                b0, b1 = bass.broadcast_tensor_aps(in0, in1)
                nc.vector.tensor_tensor(
                    t1c[:, :, 64 * h:64 * (h + 1)], b0, b1, op=mult)
            zc = zpool.tile([P, 4, P], BF16, tag="zc", name=f"zc_{c}")
            for t in reversed(range(t0, t0 + 4)):
                i = t - t0
                for h in (0, 1):
                    sl = slice(64 * h, 64 * (h + 1))
                    nc.vector.scalar_tensor_tensor(
                        zc[:, i, sl], pss[t][:, sl],
                        w_full[:, t, h:h + 1], t1c[:, i, sl],
                        op0=mult, op1=add)
            ztc = ztpool.tile([P, 4, P], BF16, tag="ztc", name=f"ztc_{c}")
            nc.sync.dma_start_transpose(ztc[:], zc[:])
            ztcs[c] = ztc

        # rotation: M(c+1) | F(c) | B(c+2).  M first so its matmuls are not
        # queued behind F's x-DMA-gated matmuls; B last so its W_O matmuls
        # consume a transpose issued a full iteration earlier (keeps PE from
        # idling >3.4us, which would re-throttle the HAM clock gate)
        for c in reversed(range(NCHUNK)):
            if c + 1 < NCHUNK:
                stage_mid(c + 1)
            stage_front(c)
            if c + 2 < NCHUNK:
                _emit_out_stage(nc, c + 2, ztcs[c + 2], pout, osb, wosb, out)
        # ---- epilogue: final chunk with pair-pipelined transpose/W_O so
        # the tail chain is [stt hi | T hi | stt lo + WO hi | T lo | WO lo].
        # Its S-matmuls/t1 are emitted BEFORE B(1) so they are not queued
        # behind B(1)'s transpose-gated W_O matmuls on the in-order PE.
        v4 = v4s[0]
        pss = {}
        for t in (3, 2, 1, 0):
            ps = psfx.tile([P, P], F32, tag="ps", name=f"ps_{t}")
            nc.tensor.matmul(ps[:], trisb[:], v4[:, t, :],
                             start=True, stop=False)
            nc.tensor.matmul(ps[:], onesb[:], R[t + 1],
                             start=False, stop=True)
            pss[t] = ps
        t1c = t1pool.tile([P, 4, P], F32, tag="t1", name="t1_ep")
        for h in (0, 1):
            b0, b1 = bass.broadcast_tensor_aps(
                v4[:, :, 64 * h:64 * (h + 1)], a_full[:, 0:4, h:h + 1])
            nc.vector.tensor_tensor(
                t1c[:, :, 64 * h:64 * (h + 1)], b0, b1, op=mult)

        _emit_out_stage(nc, 1, ztcs[1], pout, osb, wosb, out)
        zc = zpool.tile([P, 4, P], BF16, tag="zc", name="zc_ep")
        ztc_pairs = {}
        o4 = osb.tile([P, 4, DMODEL], BF16, tag="o", name="o4_ep")
        for half in (1, 0):
            for i in (2 * half + 1, 2 * half):
                for h in (0, 1):
                    sl = slice(64 * h, 64 * (h + 1))
                    nc.vector.scalar_tensor_tensor(
                        zc[:, i, sl], pss[i][:, sl],
                        w_full[:, i, h:h + 1], t1c[:, i, sl],
                        op0=mult, op1=add)
            ztc2 = ztpool.tile([P, 2, P], BF16, tag="ztc2",
                               name=f"ztc_ep_{half}")
            nc.sync.dma_start_transpose(ztc2[:], zc[:, 2 * half:2 * half + 2, :])
            ztc_pairs[half] = ztc2
            for i in (2 * half + 1, 2 * half):
                po = pout.tile([P, DMODEL], F32, tag="po", name=f"po_ep{i}")
                nc.tensor.matmul(po[:], ztc2[:, i % 2, :], wosb[:],
                                 start=True, stop=True)
                nc.vector.tensor_copy(o4[:, i, 0:192], po[:, 0:192])
                nc.scalar.copy(o4[:, i, 192:512], po[:, 192:512])
            nc.sync.dma_start(
                out[:, 2 * half:2 * half + 2, :],
                o4[:, 2 * half:2 * half + 2, :])


def _emit_out_stage(nc, c, ztc, pout, osb, wosb, out):
    o4 = osb.tile([P, 4, DMODEL], BF16, tag="o", name=f"o4_{c}")
    for t in reversed(range(4 * c, 4 * c + 4)):
        i = t - 4 * c
        po = pout.tile([P, DMODEL], F32, tag="po", name=f"po_{t}")
        nc.tensor.matmul(po[:], ztc[:, i, :], wosb[:], start=True, stop=True)
        nc.vector.tensor_copy(o4[:, i, 0:192], po[:, 0:192])
        nc.scalar.copy(o4[:, i, 192:512], po[:, 192:512])
    # one batched DMA for the whole chunk (p-major dram layout)
    t0 = 4 * c
    nc.sync.dma_start(out[:, t0:t0 + 4, :], o4[:])


_NC_CACHE = {}


def _get_nc():
    if "nc" not in _NC_CACHE:
        _NC_CACHE["nc"] = _build_nc()
    return _NC_CACHE["nc"]


def _make_in_maps(x, W_Q, W_K, W_V, W_O):
    tri = np.tril(np.ones((P, P), np.float32)).astype(NPBF16)
    ident = np.eye(P, dtype=np.float32).astype(NPBF16)
    pos = (np.arange(NT)[None, :] * P + np.arange(P)[:, None]).astype(np.float32)
    cnt1 = (SEQ - 1) - pos                      # [128, 16]
    cnt = np.stack([cnt1, cnt1], axis=2)        # [128, 16, 2]
    cnt = np.ascontiguousarray(cnt, np.float32)

    in_maps = []
    for core in range(NCORES):
        b, g = core // 4, core % 4
        xTb = np.asarray(x[b]).T.astype(NPBF16)          # [512, 2048]
        # -> [q, pd, j, c]: xq[q, pd, j, c] = xT[128j+pd, 512q+c]
        xq = np.ascontiguousarray(
            xTb.reshape(4, P, 4, DMODEL).transpose(2, 1, 0, 3))
        wq = np.asarray(W_Q[2 * g:2 * g + 2]).reshape(P, DMODEL).T
        wk = np.asarray(W_K[2 * g:2 * g + 2]).reshape(P, DMODEL).T
        wv = np.asarray(W_V[2 * g:2 * g + 2]).reshape(P, DMODEL).T
        wqkv = np.concatenate([wq, wk, wv], axis=1).astype(NPBF16)  # [512,384]
        # -> [pd, j, 384]: whost[pd, j, :] = wqkv[128j+pd, :]
        whost = np.ascontiguousarray(
            wqkv.reshape(4, P, 384).transpose(1, 0, 2))
        wo_c = np.ascontiguousarray(
            np.asarray(W_O)[:, P * g:P * (g + 1)].T).astype(NPBF16)
        in_maps.append({
            "xT": xq, "wqkv": whost, "wo": wo_c,
            "trit": tri, "ident": ident, "cnt": cnt,
        })
    return in_maps


def _run(x, W_Q, W_K, W_V, W_O, trace=False, **spmd_kwargs):
    nc = _get_nc()
    in_maps = _make_in_maps(x, W_Q, W_K, W_V, W_O)
    res = run_bass_kernel_spmd(nc, in_maps, core_ids=list(range(NCORES)),
                               trace=trace, **spmd_kwargs)
    # device output is p-major [128, 16, 512]; back to [2048, 512]
    outs = [r["out"].astype(np.float32).transpose(1, 0, 2).reshape(SEQ, DMODEL)
            for r in res.results]
    full = np.stack([
        outs[0] + outs[1] + outs[2] + outs[3],
        outs[4] + outs[5] + outs[6] + outs[7],
    ])  # [2, 2048, 512]
    return full, res


def kernel(x, W_Q, W_K, W_V, W_O):
    full, _ = _run(np.asarray(x), np.asarray(W_Q), np.asarray(W_K),
                   np.asarray(W_V), np.asarray(W_O))
    return full
